# revision 31
# baseline (speedup 1.0000x reference)
"""GAT layer kernel for Trainium2, data-parallel over batch across 8 NeuronCores.

Per batch element b (one core each):
    hp  = h @ W_proj + b_proj                      # [N, D]
    s   = hp @ w_src ; t = hp @ w_dst              # [N]
    e   = relu(s[:,None] + t[None,:] + b_att)      # [N, N]
    att = exp(e) * a ; att /= att.sum(-1, keepdim) # [N, N]
    out = att @ hp + hp                            # [N, D]

Key identities:
  exp(relu(x)) == max(exp(x), 1)     -> relu becomes a max against 1.0
  exp(s_i+t_j) == exp(s_i)*exp(t_j)  -> z = exp(t_full + s_i) via one ACT op

Schedule (the big win over the first version): the h-dependent setup chain
(hT -> hpT -> hp/s/t) runs CONCURRENTLY with the streaming a-load instead of
after it.  h and out use a pre-blocked [p, r, d] DRAM layout so their DMAs are
128 x 8KB contiguous descriptors (line rate) instead of 2048 x 512B.  t_full
replication uses a PE rank-1 broadcast (ones x t_row) instead of log-doubling
SBUF->SBUF DMAs (which serialize against the xbar transposes).  Main matmuls
use bf16 DoubleRow perf mode (2 j-blocks contracted per pass).  Finalize is
interleaved lagged one quad; out is stored per quad.
"""

import os
import sys

for _p in ("/opt/trn_rl_repo", "/root/.axon_site/_ro/trn_rl_repo"):
    if _p not in sys.path and os.path.isdir(_p):
        sys.path.append(_p)

import numpy as np
from contextlib import ExitStack

import concourse.bass as bass
import concourse.bacc as bacc
import concourse.tile as tile
from concourse import masks, mybir
from concourse.bass_utils import run_bass_kernel_spmd

F32 = mybir.dt.float32
BF16 = mybir.dt.bfloat16

B, N, D = 8, 2048, 128
P = 128           # partitions
NT = N // P       # 16 row/col blocks
N_CORES = 8

NQ = 4            # a-load quads
QB = NT // NQ     # 4 row-blocks per quad


def _build_kernel(ctx: ExitStack, tc: tile.TileContext, io: dict):
    nc = tc.nc
    a = io["a"]            # [N, N] f32 dram
    h = io["h_blk"]        # [P, NT, D] f32 dram, h_blk[p, r, d] = h[r*128+p, d]
    consts = io["consts"]  # [D, 132] f32 dram: [W | b_proj | w_src | w_dst | b_att]
    out = io["out_blk"]    # [P, NT, D] f32 dram, same blocking as h

    cst = ctx.enter_context(tc.tile_pool(name="cst", bufs=1))
    sps = ctx.enter_context(tc.tile_pool(name="sps", bufs=3, space="PSUM"))
    ops_pool = ctx.enter_context(tc.tile_pool(name="ops", bufs=1, space="PSUM"))
    tps_pool = ctx.enter_context(tc.tile_pool(name="tps", bufs=1, space="PSUM"))
    a_pool = ctx.enter_context(tc.tile_pool(name="a", bufs=1))

    # ---- a-load stream: SWDGE cast-DMA f32->bf16, one DMA per quad.
    # Issued first so Q7 descriptor generation starts immediately; 4 buffers
    # so the whole 16MB stream runs back-to-back with no reuse stalls.
    a_tiles = {}

    def emit_load(q):
        if q < NQ:
            a_t = a_pool.tile([P, QB, N], BF16, tag=f"a{q}")
            nc.gpsimd.dma_start(
                a_t[:],
                a[q * QB * P:(q + 1) * QB * P, :].rearrange(
                    "(u p) j -> p u j", p=P))
            a_tiles[q] = a_t

    for q in range(NQ):
        emit_load(q)

    # ---- small HWDGE loads: h (8KB/partition contiguous) + packed consts ----
    h_sb = cst.tile([P, NT, D], F32)
    nc.sync.dma_start(h_sb[:], h[:])
    cpack = cst.tile([P, 132], F32)
    nc.sync.dma_start(cpack[:], consts[:])
    W_sb = cpack[:, 0:D]
    bp_col = cpack[:, D:D + 1]
    ba_sb = cpack[0:1, 131:132]

    # identity: memset on DVE; the gpsimd affine_select sits after the two
    # a-quad descgen ops in the gpsimd stream, so the a-stream starts first
    ident = cst.tile([P, P], F32)
    nc.vector.memset(ident[:], 0.0)
    nc.gpsimd.affine_select(
        out=ident[:], in_=ident[:],
        compare_op=mybir.AluOpType.not_equal, fill=1.0, base=0,
        pattern=[[-1, P]], channel_multiplier=1)

    # [w_src | 0...0 | w_dst] with w_dst in column 32 (APs start at mult of 32)
    wsd_sb = cst.tile([P, 33], F32)
    nc.vector.memset(wsd_sb[:], 0.0)
    nc.vector.tensor_copy(wsd_sb[:, 0:1], cpack[:, 129:130])
    nc.vector.tensor_copy(wsd_sb[:, 32:33], cpack[:, 130:131])

    # ---- folded attention weights: s = h @ (W @ w_src) + b_proj.w_src etc.
    # lets s/t/t_full come straight from hT, keeping hpT off the critical
    # path to the first EXP.
    ones_row = cst.tile([1, P], F32)
    wfold = cst.tile([P, 2], F32)       # [:, 0] = W@w_src, [:, 1] = W@w_dst
    bfold = cst.tile([1, 2], F32)       # [b_proj.w_src, b_proj.w_dst]
    bt_sc = cst.tile([1, 1], F32)       # b_s + b_t + b_att (all on the t side)
    WT_sb = cst.tile([P, P], F32)
    ws_rep = cst.tile([P, P], F32)      # W@w_src replicated on all partitions
    with tc.high_priority():
        nc.vector.memset(ones_row[:], 1.0)
        WT_ps = sps.tile([P, 512], F32, tag="sps")
        nc.tensor.matmul(WT_ps[:, :P], W_sb, ident[:], is_transpose=True)
        nc.scalar.copy(WT_sb[:], WT_ps[:, :P])
        wf_ps = sps.tile([P, 512], F32, tag="sps")
        nc.tensor.matmul(wf_ps[:, 0:2], WT_sb[:], cpack[:, 129:131])
        nc.tensor.matmul(wf_ps[0:1, 4:6], bp_col, cpack[:, 129:131])
        # ws row form [1, in] = w_src^T @ W^T, for the partition replicate
        # (own psum tile: a start=True matmul zeroes its bank region, which
        # would clobber wf_ps results landing in the same bank)
        wsrow_ps = sps.tile([P, 512], F32, tag="sps")
        nc.tensor.matmul(wsrow_ps[0:1, :P], cpack[:, 129:130], WT_sb[:])
        nc.scalar.copy(wfold[:], wf_ps[:, 0:2])
        ws_row = cst.tile([1, P], F32)
        nc.scalar.copy(ws_row[:], wsrow_ps[0:1, :P])
        nc.vector.tensor_copy(bfold[:], wf_ps[0:1, 4:6])
        bt_tmp = cst.tile([1, 1], F32)
        nc.vector.tensor_add(bt_tmp[:], bfold[:, 1:2], ba_sb)
        nc.vector.tensor_add(bt_sc[:], bt_tmp[:], bfold[:, 0:1])
        wsr_ps = sps.tile([P, 512], F32, tag="sps")
        nc.tensor.matmul(wsr_ps[:, :P], ones_row[:], ws_row[:])
        nc.scalar.copy(ws_rep[:], wsr_ps[:, :P])

    # ---- s_col [p, r] on DVE only: per block, one stt computes
    # sum_in h[p, in] * ws'[in] via the accumulator.  No PE, no hT, no ACT.
    s_col = cst.tile([P, NT], F32)
    s_scr = cst.tile([P, P], F32)
    with tc.high_priority():
        for r in range(NT):
            nc.vector.scalar_tensor_tensor(s_scr[:], h_sb[:, r, :], 1.0,
                                           ws_rep[:],
                                           mybir.AluOpType.mult,
                                           mybir.AluOpType.mult,
                                           accum_out=s_col[:, r:r + 1])

    # ---- critical chain to the first EXP: per slab, hT transposes then
    # t_row = wt'^T @ hT (+bias) then the PE rank-1 replicate.  All high
    # priority so the list scheduler never runs bulk setup work ahead of it.
    # ps_t / ps_b borrow the finalize pool's banks (distinct tags = no
    # serialization against the hT transposes' sps rotation).
    hT = cst.tile([P, N], F32)
    t_row = cst.tile([1, N], F32)
    t_full = cst.tile([P, N], F32)
    with tc.high_priority():
        for s4 in range(4):
            sl = slice(s4 * 512, (s4 + 1) * 512)
            for rr in range(4):
                r = s4 * 4 + rr
                ps = sps.tile([P, 512], F32, tag="sps")
                nc.tensor.matmul(ps[:, :P], h_sb[:, r, :], ident[:],
                                 is_transpose=True)
                nc.scalar.copy(hT[:, r * P:(r + 1) * P], ps[:, :P])
            ps_t = tps_pool.tile([P, 512], F32, tag="tp0")
            nc.tensor.matmul(ps_t[0:1, :], wfold[:, 1:2], hT[:, sl])
            nc.scalar.activation(t_row[:, sl], ps_t[0:1, :],
                                 mybir.ActivationFunctionType.Identity,
                                 bias=bt_sc[:], scale=1.0)
            ps_b = tps_pool.tile([P, 512], F32, tag="tp1")
            nc.tensor.matmul(ps_b[:], ones_row[:], t_row[:, sl])
            nc.vector.tensor_copy(t_full[:, sl], ps_b[:])

    # ---- hpT [d, n] = (h @ W + b).T : lhsT=W [in,d], rhs=hT [in,n] ----
    # Demoted priority: only needed by the big matmuls / residual (~35us),
    # so its ACT copies must never delay the first EXPs in the ACT stream.
    hpT = cst.tile([P, N], F32)
    hp_b16 = cst.tile([P, NT, D], BF16)
    with tc.high_priority(offset=-1_000_000):
        for s4 in range(4):
            sl = slice(s4 * 512, (s4 + 1) * 512)
            ps = sps.tile([P, 512], F32, tag="sps")
            nc.tensor.matmul(ps[:], W_sb, hT[:, sl])
            nc.scalar.activation(hpT[:, sl], ps[:],
                                 mybir.ActivationFunctionType.Identity,
                                 bias=bp_col, scale=1.0)
        # hp bf16 natural [p, r, d] (matmul lhsT + residual)
        for r in range(NT):
            ps = sps.tile([P, 512], F32, tag="sps")
            nc.tensor.matmul(ps[:, :P], hpT[:, r * P:(r + 1) * P], ident[:],
                             is_transpose=True)
            nc.scalar.copy(hp_b16[:, r, :], ps[:, :P])

    # ---- main loop pools ----
    z_pool = ctx.enter_context(tc.tile_pool(name="z", bufs=1))
    pb_pool = ctx.enter_context(tc.tile_pool(name="pb", bufs=1))
    pbth_pool = ctx.enter_context(tc.tile_pool(name="pbth", bufs=1))
    rs_pool = ctx.enter_context(tc.tile_pool(name="rs", bufs=1))
    ot_pool = ctx.enter_context(tc.tile_pool(name="ot", bufs=1))

    out_stage = cst.tile([P, NT, D], F32)

    def finalize_u(o_ps, rsum, q, u):
        # oT[d, 128 i] -> out rows, scale 1/rowsum, + residual; store per pair
        r = 4 * q + u
        usl = slice(u * P, (u + 1) * P)
        oT_sb = ot_pool.tile([P, P], F32, tag=f"ot{r % 2}")
        nc.scalar.copy(oT_sb[:], o_ps[:, usl])
        tp = tps_pool.tile([P, 512], F32, tag=f"tp{r % 2}")
        nc.tensor.matmul(tp[:, :P], oT_sb[:], ident[:], is_transpose=True)
        rinv = rs_pool.tile([P, 1], F32, tag=f"ri{u % 2}")
        nc.vector.reciprocal(rinv[:], rsum[:])
        # fused scale + residual: out = tp * (1/rowsum) + hp
        nc.vector.scalar_tensor_tensor(out_stage[:, r, :], tp[:, :P], rinv[:],
                                       hp_b16[:, r, :],
                                       mybir.AluOpType.mult,
                                       mybir.AluOpType.add)
        # store on the ACT HWDGE ring: keeps the Sync ring (transposes)
        # free of head-of-line blocking behind finalize waits
        if u % 2 == 1:
            nc.scalar.dma_start(out[:, r - 1:r + 1, :],
                                out_stage[:, r - 1:r + 1, :])

    pending = []
    for q in range(NQ):
        a_t = a_tiles.pop(q)
        pbT_q = pbth_pool.tile([P, QB * NT, P], BF16, tag=f"pt{q % 2}")
        o_q = ops_pool.tile([P, QB * P], F32, tag=f"oq{q % 2}")
        rsums = []
        for u in range(QB):
            r = QB * q + u

            # z in f32: costs nothing (stt and EXP are 1x either way) but a
            # 4-byte operand makes the scheduler's cost model agree with the
            # hardware's 1x stt rate, so its lazy sem waits fire on time.
            z_t = z_pool.tile([P, N], F32, tag=f"z{r % 4}")
            nc.scalar.activation(z_t[:], t_full[:],
                                 mybir.ActivationFunctionType.Exp,
                                 bias=s_col[:, r:r + 1], scale=1.0)

            pb_u = pb_pool.tile([P, N], BF16, tag=f"pb{r % 4}")
            rsum = rs_pool.tile([P, 1], F32, tag=f"rs{r}")
            nc.vector.scalar_tensor_tensor(pb_u[:], z_t[:], 1.0,
                                           a_t[:, u, :],
                                           mybir.AluOpType.max,
                                           mybir.AluOpType.mult,
                                           accum_out=rsum[:])
            rsums.append(rsum)
            # per-row-block xbar transpose, alternating the two HWDGE rings
            # (SP and ACT) so consecutive transposes run in parallel:
            # pbT_q[j, u*16+c, i] = P[i=u-block rows, c*128+j]
            ring = nc.sync if r % 2 == 0 or True else nc.scalar
            ring.dma_start_transpose(
                out=pbT_q[:, u * NT:(u + 1) * NT, :], in_=pb_u[:])

        if q < NQ - 1:
            # 16 matmuls, N=512: rhs for chunk c gathers the 4 u-slices.
            rhs_v = pbT_q[:].rearrange("p (u c) i -> p c u i", c=NT)
            for c in range(NT):
                nc.tensor.matmul(o_q[:], hp_b16[:, c, :], rhs_v[:, c],
                                 start=(c == 0), stop=(c == NT - 1))
            for u in range(QB):
                pending.append((o_q, rsums[u], q, u))
                if len(pending) > 3:
                    finalize_u(*pending.pop(0))
        else:
            # final quad: per-u matmul groups so the tail after the last
            # transpose is one 128-col group, not a 512-col one.
            for u in range(QB):
                for c in range(NT):
                    nc.tensor.matmul(o_q[:, u * P:(u + 1) * P],
                                     hp_b16[:, c, :],
                                     pbT_q[:, u * NT + c, :],
                                     start=(c == 0), stop=(c == NT - 1))
                pending.append((o_q, rsums[u], q, u))
                if len(pending) > 3:
                    finalize_u(*pending.pop(0))

    for item in pending:
        finalize_u(*item)


_CACHE = {}


def _get_compiled():
    if "nc" in _CACHE:
        return _CACHE["nc"], _CACHE["names"]

    nc = bacc.Bacc("TRN2", target_bir_lowering=False, debug=False)
    io = {}
    io["a"] = nc.dram_tensor("a", [N, N], F32, kind="ExternalInput").ap()
    io["h_blk"] = nc.dram_tensor("h_blk", [P, NT, D], F32,
                                 kind="ExternalInput").ap()
    io["consts"] = nc.dram_tensor("consts", [D, 132], F32,
                                  kind="ExternalInput").ap()
    io["out_blk"] = nc.dram_tensor("out_blk", [P, NT, D], F32,
                                   kind="ExternalOutput").ap()

    with tile.TileContext(nc) as tc:
        with ExitStack() as ctx:
            _build_kernel(ctx, tc, io)
    nc.compile()

    _CACHE["nc"] = nc
    _CACHE["names"] = list(io.keys())
    return nc, _CACHE["names"]


def _make_in_maps(a, h, W_proj, b_proj, w_att, b_att):
    a = np.ascontiguousarray(a, dtype=np.float32)
    h = np.ascontiguousarray(h, dtype=np.float32)
    # blocked layout: h_blk[p, r, d] = h[r*128+p, d]
    h_blk = np.ascontiguousarray(
        h.reshape(B, NT, P, D).transpose(0, 2, 1, 3))
    consts = np.zeros((D, 132), dtype=np.float32)
    consts[:, 0:D] = np.asarray(W_proj, dtype=np.float32)
    consts[:, D] = np.asarray(b_proj, dtype=np.float32)
    w_att = np.ascontiguousarray(w_att, dtype=np.float32)
    consts[:, 129] = w_att[:D]
    consts[:, 130] = w_att[D:]
    consts[:, 131] = np.float32(b_att)

    in_maps = []
    for c in range(N_CORES):
        in_maps.append({"a": a[c], "h_blk": h_blk[c], "consts": consts})
    return in_maps


def _get_executable():
    """Build (once) a sharded PJRT callable for the compiled Bass module.

    Mirrors concourse.bass2jax.run_bass_via_pjrt but keeps the jitted
    function so repeated calls don't retrace/recompile.
    """
    if "exe" in _CACHE:
        return _CACHE["exe"]

    import jax
    from jax.sharding import Mesh, PartitionSpec
    from jax.experimental.shard_map import shard_map
    from concourse import bass2jax, mybir as _mybir

    nc, _ = _get_compiled()
    bass2jax.install_neuronx_cc_hook()

    partition_name = (nc.partition_id_tensor.name
                      if nc.partition_id_tensor else None)
    in_names, out_names, out_avals, zero_outs = [], [], [], []
    for alloc in nc.m.functions[0].allocations:
        if not isinstance(alloc, _mybir.MemoryLocationSet):
            continue
        name = alloc.memorylocations[0].name
        if alloc.kind == "ExternalInput":
            if name != partition_name:
                in_names.append(name)
        elif alloc.kind == "ExternalOutput":
            shape = tuple(alloc.tensor_shape)
            dtype = _mybir.dt.np(alloc.dtype)
            out_names.append(name)
            out_avals.append(jax.core.ShapedArray(shape, dtype))
            zero_outs.append(np.zeros(shape, dtype))
    n_params = len(in_names)
    n_outs = len(out_avals)
    all_in_names = in_names + out_names + (
        [partition_name] if partition_name else [])
    donate = tuple(range(n_params, n_params + n_outs))

    def _body(*args):
        operands = list(args)
        if partition_name is not None:
            operands.append(bass2jax.partition_id_tensor())
        outs = bass2jax._bass_exec_p.bind(
            *operands,
            out_avals=tuple(out_avals),
            in_names=tuple(all_in_names),
            out_names=tuple(out_names),
            lowering_input_output_aliases=(),
            sim_require_finite=True,
            sim_require_nnan=True,
            nc=nc,
        )
        return tuple(outs)

    devices = jax.devices()[:N_CORES]
    mesh = Mesh(np.asarray(devices), ("core",))
    in_specs = (PartitionSpec("core"),) * (n_params + n_outs)
    out_specs = (PartitionSpec("core"),) * n_outs
    fn = jax.jit(
        shard_map(_body, mesh=mesh, in_specs=in_specs, out_specs=out_specs,
                  check_rep=False),
        donate_argnums=donate, keep_unused=True,
    )
    exe = {
        "fn": fn, "mesh": mesh, "in_names": in_names,
        "out_names": out_names, "out_avals": out_avals,
        "zero_outs": zero_outs, "n_params": n_params,
    }
    _CACHE["exe"] = exe
    return exe


def _concat_inputs(exe, in_maps):
    return [
        np.concatenate([np.asarray(in_maps[c][name])
                        for c in range(N_CORES)], axis=0)
        for name in exe["in_names"]
    ]


def _concat_zeros(exe):
    return [np.zeros((N_CORES * z.shape[0], *z.shape[1:]), z.dtype)
            for z in exe["zero_outs"]]


def kernel(a, h, W_proj, b_proj, w_att, b_att):
    exe = _get_executable()
    in_maps = _make_in_maps(a, h, W_proj, b_proj, w_att, b_att)
    out_arrs = exe["fn"](*_concat_inputs(exe, in_maps), *_concat_zeros(exe))
    i = exe["out_names"].index("out_blk")
    out_blk = np.asarray(out_arrs[i]).reshape(N_CORES, P, NT, D)
    # un-block: out[n, d] = out_blk[n%128, n//128, d]
    return np.ascontiguousarray(out_blk.transpose(0, 2, 1, 3)).reshape(
        N_CORES, N, D)


if __name__ == "__main__":
    rng = np.random.default_rng(0)
    a = rng.random((B, N, N), dtype=np.float32)
    h = rng.standard_normal((B, N, D)).astype(np.float32)
    W_proj = (rng.standard_normal((D, D)) / np.sqrt(D)).astype(np.float32)
    b_proj = (rng.standard_normal(D) * 0.01).astype(np.float32)
    w_att = (rng.standard_normal(2 * D) / np.sqrt(2 * D)).astype(np.float32)
    b_att = np.float32(rng.standard_normal() * 0.01)

    got = kernel(a=a, h=h, W_proj=W_proj, b_proj=b_proj, w_att=w_att,
                 b_att=b_att)

    hp = h @ W_proj + b_proj
    s = hp @ w_att[:D]
    t = hp @ w_att[D:]
    e = np.maximum(s[:, :, None] + t[:, None, :] + b_att, 0.0)
    att = np.exp(e) * a
    att = att / att.sum(-1, keepdims=True)
    ref = att @ hp + hp

    err = np.abs(got - ref).max() / np.abs(ref).max()
    print("rel err:", err)
